# revision 12
# baseline (speedup 1.0000x reference)
"""BailingMoe (T=1024, H=1024, E=16, K=4, I=1408, IS=2816) on 8 TRN2 cores.

Strategy: expert-parallel, 2 experts per core, balanced pairing (largest
expert with smallest). The router (x @ w_gate, softmax, top-4, renorm -
0.02% of FLOPs) runs on host as part of input sharding: tokens are
gathered per expert into two capacity slots sized EXACTLY to the max
big / max small expert load (no rounding - matmul free dims are
arbitrary). Each core computes its two experts' MLPs on their gathered
tokens (bf16 matmuls, f32 PSUM accumulation) and a tensor-parallel
slice (IS/8 = 352, zero-padded to 384) of the shared expert over all
tokens. Host unshards: scatter-add the weighted expert outputs and sum
the 8 shared-expert partials.

v3 (after trace analysis of the 124us v2):
- The v2 front half was DMA-starved (saturated 360 GB/s while the PE
  idled 4.3us and HAM re-throttled to half clock for 13.5us). v3 issues
  every input DMA up front in strict need-order on dedicated queues:
  sync=x tiles, scalar=shared weights + gathered x, gpsimd=bulk routed
  weights (gated behind the last shared-weight arrival so the urgent
  queues get full bandwidth first). Pool rotation (bufs=1 on the bulk
  tags) self-paces slot-1 weight DMAs behind slot-0 consumption.
- Big DMAs: p-major DRAM layouts so each transfer is one contiguous
  multi-MB descriptor (wgu in 3 chunks/slot, wdn in 1 chunk per
  (slot,half)): ~25 input DMA issues instead of ~90 (each costs ~635ns
  of engine time).
- Sequential phases in PE program order: warm-up MMs on zeros (keeps
  the HAM clock-gate open from t=0), shared gate_up, shared down,
  routed gu0, dn0, gu1, dn1. Down passes are bank-major (4 chains of
  11 accumulating MMs) so each bank's combine-mul + output DMA overlaps
  the next chain; no epilogue tail pileup.
- PSUM: psg/psu 2-deep for all gate_up chains, 4 banks rotating for all
  down/shared-down chains (8 banks total).
"""

import functools

import numpy as np
import ml_dtypes

T = 1024
H = 1024
E = 16
K = 4
I = 1408
IS = 2816
ISP = 384          # padded per-core shared-expert slice (2816/8 = 352 -> 384)
TI = I // 128      # 11 intermediate col/row tiles per routed expert
N_CORES = 8

BF16 = ml_dtypes.bfloat16

GU_CHUNKS = (4, 4, 3)  # TI=11 split for the big gate_up weight DMAs


def _build_nc(C0: int, C1: int):
    import concourse.bass as bass  # noqa: F401  (bacc needs bass loaded)
    import concourse.mybir as mybir
    import concourse.tile as tile
    from concourse import bacc

    BF = mybir.dt.bfloat16
    F32 = mybir.dt.float32
    CS = C0 + C1

    nc = bacc.Bacc(None, target_bir_lowering=False, debug=False)

    # All bulk inputs are pre-tiled on host so every DMA is contiguous.
    xT_ext = nc.declare_dram_parameter("xT", [2, 2, 128, 4, 512], BF, isOutput=False)
    xe0_ext = nc.declare_dram_parameter("xe0", [128, 8, C0], BF, isOutput=False)
    xe1_ext = nc.declare_dram_parameter("xe1", [128, 8, C1], BF, isOutput=False)
    wtb_ext = nc.declare_dram_parameter("wtb", [128, CS], F32, isOutput=False)
    wgu_ext = nc.declare_dram_parameter(
        "w_gu", [2, 128, TI, 2, 8, 128], BF, isOutput=False
    )
    wdn_ext = nc.declare_dram_parameter(
        "w_dn", [2, 128, 2, TI, 512], BF, isOutput=False
    )
    wsgu_ext = nc.declare_dram_parameter(
        "w_sgu", [3, 128, 2, 8, 128], BF, isOutput=False
    )
    wsd_ext = nc.declare_dram_parameter("w_sd", [128, 2, 3, 512], BF, isOutput=False)
    out_ext = nc.declare_dram_parameter("out", [128, 8, CS + T], BF, isOutput=True)

    SILU = mybir.ActivationFunctionType.Silu

    with tile.TileContext(nc) as tc:
        with (
            tc.tile_pool(name="xpool", bufs=1) as xpool,
            tc.tile_pool(name="wp", bufs=1) as wp,
            tc.tile_pool(name="wgu_pool", bufs=1) as wgu_pool,
            tc.tile_pool(name="wdn_pool", bufs=1) as wdn_pool,
            tc.tile_pool(name="a_pool", bufs=1) as a_pool,
            tc.tile_pool(name="tmp_pool", bufs=3) as tmp_pool,
            tc.tile_pool(name="ysd_pool", bufs=2) as ysd_pool,
            tc.tile_pool(name="ydn_pool", bufs=3) as ydn_pool,
            tc.tile_pool(name="pg", bufs=2, space="PSUM") as pg,
            tc.tile_pool(name="pd", bufs=4, space="PSUM") as pd,
        ):
            # ---- input DMAs, issued eagerly in strict need-order.
            # sync queue (shortest preamble): everything the shared gate_up
            # phase touches, in exact consumption order.
            xsb = [
                [
                    xpool.tile(
                        [128, 4, 512], BF, tag=f"x{t}{hf}", name=f"xsb{t}{hf}"
                    )
                    for hf in range(2)
                ]
                for t in range(2)
            ]
            wsgu_sb = [
                wp.tile([128, 2, 8, 128], BF, tag=f"wsgu{j}", name=f"wsgu{j}")
                for j in range(3)
            ]
            nc.sync.dma_start(wsgu_sb[0][:], wsgu_ext[0])
            nc.sync.dma_start(xsb[0][0][:], xT_ext[0, 0])
            nc.sync.dma_start(xsb[0][1][:], xT_ext[0, 1])
            nc.sync.dma_start(wsgu_sb[1][:], wsgu_ext[1])
            nc.sync.dma_start(wsgu_sb[2][:], wsgu_ext[2])
            nc.sync.dma_start(xsb[1][0][:], xT_ext[1, 0])
            nc.sync.dma_start(xsb[1][1][:], xT_ext[1, 1])
            # gpsimd queue: everything else. The tile scheduler reorders
            # same-engine instructions by readiness, so a single blocked
            # "gate" op cannot fence the queue; instead every first-wave bulk
            # tile gets a real dependency: a 1-column copy from wsgu1 written
            # into the tile before its DMA (the overlapping-write hazard
            # orders the DMA after the gate, and the gate waits for wsgu1's
            # arrival - by which point the urgent sync queue has drained).
            gate_src = wsgu_sb[1][:, 0, 0, :1]

            def gated_dma(t, gslice, src):
                nc.gpsimd.tensor_copy(gslice, gate_src)
                nc.gpsimd.dma_start(t[:], src)

            wsd_sb = wp.tile([128, 2, 3, 512], BF, tag="wsd")
            xesb0 = xpool.tile([128, 8, C0], BF, tag="xe0")
            xesb1 = xpool.tile([128, 8, C1], BF, tag="xe1")
            wtb_sb = xpool.tile([128, CS], F32, tag="wtb")
            gated_dma(wsd_sb, wsd_sb[:, 0, 0, :1], wsd_ext[:])
            gated_dma(xesb0, xesb0[:, 0, :1], xe0_ext[:])
            gated_dma(xesb1, xesb1[:, 0, :1], xe1_ext[:])
            gated_dma(wtb_sb, wtb_sb[:, :1], wtb_ext[:])

            wgu_tiles = ([], [])   # per slot: 3 chunk tiles
            wdn_tiles = ([], [])   # per slot: 2 half tiles

            def issue_slot_weights(s):
                off = 0
                for ci, cs_ in enumerate(GU_CHUNKS):
                    t = wgu_pool.tile(
                        [128, cs_, 2, 8, 128], BF, tag=f"wgu{ci}", name=f"wgu{s}{ci}"
                    )
                    if s == 0:
                        gated_dma(t, t[:, 0, 0, 0, :1], wgu_ext[s][:, off : off + cs_])
                    else:
                        nc.gpsimd.dma_start(t[:], wgu_ext[s][:, off : off + cs_])
                    wgu_tiles[s].append(t)
                    off += cs_
                for hh in range(2):
                    t = wdn_pool.tile(
                        [128, TI, 512], BF, tag=f"wdn{hh}", name=f"wdn{s}{hh}"
                    )
                    if s == 0:
                        gated_dma(t, t[:, 0, :1], wdn_ext[s][:, hh])
                    else:
                        nc.gpsimd.dma_start(t[:], wdn_ext[s][:, hh])
                    wdn_tiles[s].append(t)

            issue_slot_weights(0)
            issue_slot_weights(1)  # bufs=1 tags: gated on slot-0 consumption

            acts = a_pool.tile([128, 3, 1024], BF, tag="acts")
            acte0 = a_pool.tile([128, TI, C0], BF, tag="a0")
            acte1 = a_pool.tile([128, TI, C1], BF, tag="a1")

            # ---- shared gate_up: 12 (tch, j) chain pairs, N=512. tch-outer
            # matches the linear DMA arrival order (xt0, wsgu0-2, xt1).
            for tch in range(2):
                for j in range(3):
                    psg = pg.tile([128, 512], F32, tag="psg", name="psg")
                    psu = pg.tile([128, 512], F32, tag="psu", name="psu")
                    for h in range(8):
                        nc.tensor.matmul(
                            psg, wsgu_sb[j][:, 0, h, :], xsb[tch][h // 4][:, h % 4, :],
                            start=(h == 0), stop=(h == 7),
                        )
                    for h in range(8):
                        nc.tensor.matmul(
                            psu, wsgu_sb[j][:, 1, h, :], xsb[tch][h // 4][:, h % 4, :],
                            start=(h == 0), stop=(h == 7),
                        )
                    tmp = tmp_pool.tile([128, 512], F32, tag="tmp", name="tmp")
                    nc.scalar.activation(tmp, psg, SILU)
                    nc.vector.tensor_mul(
                        acts[:, j, tch * 512 : (tch + 1) * 512], tmp, psu
                    )

            # ---- shared down: bank-major, 16 chains of 3 accumulating MMs.
            for tch in range(2):
                for hh in range(2):
                    yv = ysd_pool.tile([128, 2, 512], BF, tag="yv", name="yv")
                    ys = ysd_pool.tile([128, 2, 512], BF, tag="ys", name="ys")
                    for b in range(4):
                        ps = pd.tile([128, 512], F32, tag="pd", name="pd")
                        for io in range(3):
                            nc.tensor.matmul(
                                ps,
                                wsd_sb[:, hh, io, b * 128 : (b + 1) * 128],
                                acts[:, io, tch * 512 : (tch + 1) * 512],
                                start=(io == 0), stop=(io == 2),
                            )
                        if b < 2:
                            nc.vector.tensor_copy(yv[:, b, :], ps)
                        else:
                            nc.scalar.copy(ys[:, b - 2, :], ps)
                    cs0 = CS + tch * 512
                    nc.sync.dma_start(
                        out_ext[:, hh * 4 : hh * 4 + 2, cs0 : cs0 + 512], yv
                    )
                    nc.scalar.dma_start(
                        out_ext[:, hh * 4 + 2 : hh * 4 + 4, cs0 : cs0 + 512], ys
                    )

            # ---- routed phases.
            def gu_phase(s, xe, Cc, a):
                for i in range(TI):
                    ci, il = divmod(i, 4)
                    wt = wgu_tiles[s][ci]
                    psg = pg.tile([128, 512], F32, tag="psg", name="psg")[:, :Cc]
                    psu = pg.tile([128, 512], F32, tag="psu", name="psu")[:, :Cc]
                    for h in range(8):
                        nc.tensor.matmul(
                            psg, wt[:, il, 0, h, :], xe[:, h, :],
                            start=(h == 0), stop=(h == 7),
                        )
                    for h in range(8):
                        nc.tensor.matmul(
                            psu, wt[:, il, 1, h, :], xe[:, h, :],
                            start=(h == 0), stop=(h == 7),
                        )
                    tmp = tmp_pool.tile([128, 512], F32, tag="tmp", name="tmp")[
                        :, :Cc
                    ]
                    nc.scalar.activation(tmp, psg, SILU)
                    nc.vector.tensor_mul(a[:, i, :], tmp, psu)

            def dn_phase(s, a, Cc, cb):
                for hh in range(2):
                    wdn = wdn_tiles[s][hh]
                    for b in range(4):
                        ps = pd.tile([128, 512], F32, tag="pd", name="pd")[:, :Cc]
                        for io in range(TI):
                            nc.tensor.matmul(
                                ps,
                                wdn[:, io, b * 128 : (b + 1) * 128],
                                a[:, io, :],
                                start=(io == 0), stop=(io == TI - 1),
                            )
                        y = ydn_pool.tile([128, 512], BF, tag="ydn", name="ydn")[
                            :, :Cc
                        ]
                        nc.vector.tensor_mul(y, ps, wtb_sb[:, cb : cb + Cc])
                        nc.sync.dma_start(
                            out_ext[:, hh * 4 + b, cb : cb + Cc], y
                        )

            gu_phase(0, xesb0, C0, acte0)
            dn_phase(0, acte0, C0, 0)
            gu_phase(1, xesb1, C1, acte1)
            dn_phase(1, acte1, C1, C0)

    nc.compile()
    return nc


@functools.lru_cache(maxsize=4)
def _compiled(C0: int, C1: int):
    return _build_nc(C0, C1)


def _route(x, w_gate):
    """Mirror the reference router: softmax, top-4 (desc, ties -> lower
    index), renormalize."""
    logits = x @ w_gate  # f32 [T, E]
    m = logits.max(axis=-1, keepdims=True)
    p = np.exp(logits - m)
    p /= p.sum(axis=-1, keepdims=True)
    order = np.argsort(-p, axis=-1, kind="stable")[:, :K]  # [T, K]
    topw = np.take_along_axis(p, order, axis=-1)
    topw = topw / topw.sum(axis=-1, keepdims=True)
    return order, topw


def kernel(hidden_states, w_gate, w_moe_gate_up, w_moe_down,
           w_shared_gate_up, w_shared_down):
    from concourse.bass_utils import run_bass_kernel_spmd

    x = np.asarray(hidden_states, dtype=np.float32)
    w_gate = np.asarray(w_gate, dtype=np.float32)
    w_moe_gate_up = np.asarray(w_moe_gate_up, dtype=np.float32)
    w_moe_down = np.asarray(w_moe_down, dtype=np.float32)
    w_shared_gate_up = np.asarray(w_shared_gate_up, dtype=np.float32)
    w_shared_down = np.asarray(w_shared_down, dtype=np.float32)

    topk_ids, topk_w = _route(x, w_gate)

    # per-expert token lists + combine weights
    rows_e = []
    wts_e = []
    for e in range(E):
        r, k = np.nonzero(topk_ids == e)
        rows_e.append(r)
        wts_e.append(topk_w[r, k].astype(np.float32))
    counts = np.array([len(r) for r in rows_e])

    # balanced pairing: sort desc; core c gets (big[c], small[c])
    order = np.argsort(-counts, kind="stable")
    slot_experts = [
        (int(order[c]), int(order[2 * N_CORES - 1 - c])) for c in range(N_CORES)
    ]
    C0 = max(16, int(max(counts[a] for a, _ in slot_experts)))
    C1 = max(16, int(max(counts[b] for _, b in slot_experts)))
    CS = C0 + C1
    assert C0 <= 512 and C1 <= 512, (C0, C1)

    nc = _compiled(C0, C1)

    xT_bf = np.ascontiguousarray(x.T).astype(BF16)  # [H, T]
    # [H, T] -> [2(tch), 2(half), 128(p), 4(h), 512]
    xT_t = np.ascontiguousarray(
        xT_bf.reshape(2, 4, 128, 2, 512).transpose(3, 0, 2, 1, 4)
    )
    # [E, H, 2I] -> [E, 128(p), 11(i), 2(g/u), 8(h), 128(c)]
    w_gu_t = np.ascontiguousarray(
        w_moe_gate_up.astype(BF16)
        .reshape(E, 8, 128, 2, TI, 128)
        .transpose(0, 2, 4, 3, 1, 5)
    )
    # [E, I, H] -> [E, 128(ip), 2(hh), 11(io), 512]
    w_dn_t = np.ascontiguousarray(
        w_moe_down.astype(BF16).reshape(E, TI, 128, 2, 512).transpose(0, 2, 3, 1, 4)
    )

    S = IS // N_CORES  # 352
    in_maps = []
    for c in range(N_CORES):
        wtb = np.zeros((CS,), dtype=np.float32)
        wgu = np.empty((2,) + w_gu_t.shape[1:], dtype=BF16)
        wdn = np.empty((2,) + w_dn_t.shape[1:], dtype=BF16)
        xes = []
        for s, e in enumerate(slot_experts[c]):
            cnt = counts[e]
            Cc = (C0, C1)[s]
            b = 0 if s == 0 else C0
            xe = np.zeros((H, Cc), dtype=BF16)
            xe[:, :cnt] = xT_bf[:, rows_e[e]]
            xes.append(
                np.ascontiguousarray(
                    xe.reshape(8, 128, Cc).transpose(1, 0, 2)
                )
            )
            wtb[b : b + cnt] = wts_e[e]
            wgu[s] = w_gu_t[e]
            wdn[s] = w_dn_t[e]
        wsgu = np.zeros((H, 2 * ISP), dtype=BF16)
        wsgu[:, :S] = w_shared_gate_up[:, c * S : (c + 1) * S].astype(BF16)
        wsgu[:, ISP : ISP + S] = w_shared_gate_up[
            :, IS + c * S : IS + (c + 1) * S
        ].astype(BF16)
        # [H, 2*ISP] -> [3(j), 128(p), 2(g/u), 8(o), 128(c)]
        wsgu_t = np.ascontiguousarray(
            wsgu.reshape(8, 128, 2, 3, 128).transpose(3, 1, 2, 0, 4)
        )
        wsd = np.zeros((ISP, H), dtype=BF16)
        wsd[:S] = w_shared_down[c * S : (c + 1) * S].astype(BF16)
        # [ISP, H] -> [128(ip), 2(hh), 3(io), 512]
        wsd_t = np.ascontiguousarray(
            wsd.reshape(3, 128, 2, 512).transpose(1, 2, 0, 3)
        )
        in_maps.append(
            {
                "xT": xT_t,
                "xe0": xes[0],
                "xe1": xes[1],
                "wtb": np.ascontiguousarray(
                    np.broadcast_to(wtb[None, :], (128, CS))
                ),
                "w_gu": wgu,
                "w_dn": wdn,
                "w_sgu": wsgu_t,
                "w_sd": wsd_t,
            }
        )

    res = run_bass_kernel_spmd(nc, in_maps, core_ids=list(range(N_CORES)))

    out = np.zeros((T, H), dtype=np.float32)
    acc = np.zeros((H, T), dtype=np.float32)
    for c in range(N_CORES):
        r = np.asarray(res.results[c]["out"], dtype=np.float32)  # [128,8,CS+T]
        r = r.transpose(1, 0, 2).reshape(H, CS + T)
        for s, e in enumerate(slot_experts[c]):
            cnt = counts[e]
            b = 0 if s == 0 else C0
            out[rows_e[e]] += r[:, b : b + cnt].T
        acc += r[:, CS:]
    out += acc.T
    return out
